# revision 15
# baseline (speedup 1.0000x reference)
"""Trainium2 Bass kernel for nn_Bottleneck_75325136437765 (sparse 3x3 local attention bottleneck).

Sharding: data-parallel over batch B=16 across 8 cores (2 batches/core), params replicated.

v2 design ("broadcast logits", no SBUF->SBUF broadcast DMA):

  Pair-interleaved channel layout for q/k/v: partition p holds channels
  chan(p,l) = 8*(p//4) + 2*(p%4) + l for l in {0,1}; head g = p//4. All 32 heads
  live in one 128-partition tile, so per-head ops never need a partition broadcast
  across tiles. Weight-column permutation on the host makes this free.

  logits: tmp_l = (k_shift + pos) * q per l (fused DVE scalar_tensor_tensor),
          padd = tmp_0 + tmp_1  (partial d-reduce in free dim)
          L_bc[p'] = sum_{p: g(p)=g(p')} padd[p]   (PE matmul, S4 block-diag 0/1)
      -> logits come out of PE already replicated 4x per head; exp (ACT) writes
         e_bc straight to SBUF. No packed layout, no expand, no broadcast DMA.
  den: running DVE adds over e_bc tiles; recip = reciprocal_approx_fast.
  v: vp_kk = e_bc[kk] (stride-0 over l) * v_shift (DVE 2x); sum over kk via
     PE identity-matmul PSUM accumulation; t3 = acc * recip (DVE);
     h2 = relu(t3 + bnatt_b) (ACT).
  conv1/conv3: plain bf16 matmuls, residual added as identity matmul on bf16 x.
  DMA: x and out in p-major layout (16KB contiguous per partition), out in bf16.
"""

import numpy as np

import concourse.bass as bass
import concourse.bacc as bacc
import concourse.tile as tile
from concourse import mybir
from concourse.bass_utils import run_bass_kernel_spmd

# ---- problem constants (hardcoded per contract) ----
B, CIN, H, W = 16, 1024, 32, 32
WIDTH, OUT, HEADS, KS = 256, 1024, 32, 3
D = WIDTH // HEADS            # 8 channels per head
HW = H * W                    # 1024
NC_ = 8                       # cores
BL = B // NC_                 # 2 batches per core
P = 128
KC1 = CIN // P                # 8 contraction chunks for conv1
PT = WIDTH // P               # 2 partition tiles for width-256 tensors
OC = OUT // P                 # 8 output ptiles for conv3
NKK = KS * KS                 # 9 shifts
F32 = mybir.dt.float32
BF16 = mybir.dt.bfloat16
NHALF = 2                     # PSUM-bank limit: matmul N<=512 fp32 out
HP = H + 2                    # padded spatial
WP = W + 2


def _ns(n):
    return slice(n * 512, (n + 1) * 512)


def build_program():
    nc = bacc.Bacc(None, target_bir_lowering=False, debug=False)

    def din(name, shape, dt=BF16):
        return nc.dram_tensor(name, list(shape), dt, kind="ExternalInput").ap()

    x16_d = din("x16", (BL, P, KC1 * HW))          # p-major for 16KB descriptors
    w1T_d = din("w1T", (KC1, P, WIDTH))
    wqT_d = din("wqT", (PT, P, PT, P))             # [kc, p, l, cols]
    wkT_d = din("wkT", (PT, P, PT, P))
    wvT_d = din("wvT", (PT, P, PT, P))
    w3T_d = din("w3T", (PT, P, OUT))               # [l-chunk, p, out]
    b1_d = din("b1", (PT, P, 1), F32)
    bq_d = din("bq", (P, PT), F32)                 # [p, l]
    bk_d = din("bk", (P, PT), F32)
    bv_d = din("bv", (P, PT), F32)
    batt_d = din("batt", (P, PT), F32)
    b3_d = din("b3", (OC, P, 1), F32)
    pos2_d = din("pos2", (P, PT, NKK), F32)        # [p, l, kk]
    s4_d = din("s4", (P, P))                       # block-diag head map
    ident_d = din("ident", (P, P))
    out_d = nc.dram_tensor("out", [BL, P, OC * HW], BF16, kind="ExternalOutput").ap()

    with tile.TileContext(nc) as tc:
        with (
            tc.tile_pool(name="consts", bufs=1) as consts,
            tc.tile_pool(name="xb", bufs=2) as xbp,
            tc.tile_pool(name="act", bufs=2) as actp,
            tc.tile_pool(name="att", bufs=1) as attp,
            tc.tile_pool(name="ebcp", bufs=3) as ebcp,
            tc.tile_pool(name="vpp", bufs=3) as vpp,
            tc.tile_pool(name="tmp", bufs=3) as tmpp,
            tc.tile_pool(name="outz", bufs=2) as outzp,
            tc.tile_pool(name="pmm", bufs=2, space="PSUM") as pmm,
            tc.tile_pool(name="pL", bufs=1, space="PSUM") as pLp,
            tc.tile_pool(name="pacc", bufs=2, space="PSUM") as paccp,
        ):
            # ---- load constants ----
            # most constants go on the gpsimd SWDGE queue so the sync queue
            # serves conv1's x/w chunks first (fast kernel start)
            def cload(name, dram, shape, dt=BF16, re="k p m -> p k m"):
                t = consts.tile(shape, dt, tag=name)
                nc.gpsimd.dma_start(out=t, in_=dram.rearrange(re) if re else dram)
                return t

            w1T = consts.tile([P, KC1, WIDTH], BF16, tag="w1T")
            b1 = consts.tile([P, PT, 1], F32, tag="b1")
            nc.sync.dma_start(out=b1, in_=b1_d.rearrange("k p m -> p k m"))
            wqT = cload("wqT", wqT_d, [P, PT, PT, P], re="k p l m -> p k l m")
            wkT = cload("wkT", wkT_d, [P, PT, PT, P], re="k p l m -> p k l m")
            wvT = cload("wvT", wvT_d, [P, PT, PT, P], re="k p l m -> p k l m")
            w3T = cload("w3T", w3T_d, [P, PT, OUT], re="k p m -> p k m")
            bq = cload("bq", bq_d, [P, PT], F32, re=None)
            bk = cload("bk", bk_d, [P, PT], F32, re=None)
            bv = cload("bv", bv_d, [P, PT], F32, re=None)
            batt = cload("batt", batt_d, [P, PT], F32, re=None)
            b3 = cload("b3", b3_d, [P, OC, 1], F32, re="k p m -> p k m")
            pos2 = cload("pos2", pos2_d, [P, PT, NKK], F32, re=None)
            s4 = cload("s4", s4_d, [P, P], re=None)
            ident = cload("ident", ident_d, [P, P], re=None)

            # per-batch zero-padded k/v tiles (double-buffered across batches)
            kpads, vpads = [], []
            for i in range(BL):
                kpad_i = consts.tile([P, PT, HP, WP], BF16, name=f"kpad{i}")
                vpad_i = consts.tile([P, PT, HP, WP], BF16, name=f"vpad{i}")
                nc.vector.memset(kpad_i, 0.0)
                nc.vector.memset(vpad_i, 0.0)
                kpads.append(kpad_i)
                vpads.append(vpad_i)

            for b in range(BL):
                kpad, vpad = kpads[b], vpads[b]
                # ---- load x (bf16, p-major, 4 chunks so conv1 starts early) ----
                xb = xbp.tile([P, KC1, HW], BF16, tag="xb")
                if b == 0:
                    nc.scalar.dma_start(out=w1T, in_=w1T_d.rearrange("k p m -> p k m"))
                for ch in range(4):
                    nc.sync.dma_start(
                        out=xb[:, 2 * ch:2 * ch + 2, :].rearrange("p k m -> p (k m)"),
                        in_=x16_d[b, :, 2 * ch * HW:(2 * ch + 2) * HW])

                # ---- conv1: h1 = relu(x @ w1' + b1) ----
                h1 = actp.tile([P, PT, HW], BF16, tag="h1")
                for mc in range(PT):
                    for n in range(NHALF):
                        ps = pmm.tile([P, 512], F32, tag="mm")
                        for kc in range(KC1):
                            nc.tensor.matmul(
                                ps,
                                w1T[:, kc, mc * P:(mc + 1) * P],
                                xb[:, kc, _ns(n)],
                                start=(kc == 0), stop=(kc == KC1 - 1),
                            )
                        nc.scalar.activation(
                            out=h1[:, mc, _ns(n)], in_=ps,
                            func=mybir.ActivationFunctionType.Relu,
                            bias=b1[:, mc], scale=1.0,
                        )

                # ---- q/k/v convs in pair-interleaved layout ----
                # chunk l of conv X: out partition p <- channel chan(p,l)
                q2 = actp.tile([P, PT, HW], BF16, tag="q2")
                for wT, bias, relu, dest in (
                    (wqT, bq, True, None),      # q2[:, l, :]
                    (wkT, bk, True, kpad),      # kpad interior
                    (wvT, bv, False, vpad),     # vpad interior
                ):
                    for l in range(PT):
                        for n in range(NHALF):
                            ps = pmm.tile([P, 512], F32, tag="mm")
                            for kc in range(PT):
                                nc.tensor.matmul(
                                    ps,
                                    wT[:, kc, l, :],
                                    h1[:, kc, _ns(n)],
                                    start=(kc == 0), stop=(kc == PT - 1),
                                )
                            if dest is None:
                                o, i = q2[:, l, _ns(n)], ps[:]
                            else:
                                o = dest[:, l, 1 + 16 * n:17 + 16 * n, 1:W + 1]
                                i = ps.rearrange("p (a b) -> p a b", a=16)
                            nc.scalar.activation(
                                out=o, in_=i,
                                func=(mybir.ActivationFunctionType.Relu if relu
                                      else mybir.ActivationFunctionType.Identity),
                                bias=bias[:, l:l + 1], scale=1.0,
                            )

                # ---- attention: per kk: logits -> exp -> den-add, v-product,
                #      and v-sum accumulation (both l) interleaved in PSUM ----
                den = attp.tile([P, HW], BF16, tag="den")
                accs = []
                for l in range(PT):
                    acc_l = paccp.tile([P, HW], F32, tag="acc", name=f"acc{l}")
                    accs.append(acc_l)
                for kk in range(NKK):
                    di, dj = kk // KS, kk % KS
                    # kpos_l = k_shift + pos (DVE tensor_scalar 4x);
                    # tmp = kpos * q (2x); padd = tmp_0 + tmp_1
                    kpos = tmpp.tile([P, PT, HW], BF16, tag="kpos")
                    for l in range(PT):
                        nc.vector.tensor_scalar_add(
                            out=kpos[:, l, :].rearrange("p (a b) -> p a b", a=H),
                            in0=kpad[:, l, di:di + H, dj:dj + W],
                            scalar1=pos2[:, l, kk:kk + 1],
                        )
                    tmp = tmpp.tile([P, PT, HW], BF16, tag="tmp")
                    nc.vector.tensor_tensor(
                        out=tmp, in0=kpos, in1=q2, op=mybir.AluOpType.mult,
                    )
                    padd = tmpp.tile([P, HW], BF16, tag="padd")
                    nc.vector.tensor_tensor(
                        out=padd, in0=tmp[:, 0, :], in1=tmp[:, 1, :],
                        op=mybir.AluOpType.add,
                    )
                    # L_bc = S4 @ padd  (head-reduce, output replicated 4x)
                    Lbc = pLp.tile([P, HW], F32, tag="Lbc")
                    for n in range(NHALF):
                        nc.tensor.matmul(
                            Lbc[:, _ns(n)], s4, padd[:, _ns(n)],
                            start=True, stop=True,
                            skip_group_check=True,
                        )
                    ebc = ebcp.tile([P, HW], BF16, tag="ebc")
                    nc.scalar.activation(
                        out=ebc, in_=Lbc,
                        func=mybir.ActivationFunctionType.Exp,
                    )
                    # den running sum (DVE bf16 2x)
                    if kk == 0:
                        nc.vector.tensor_scalar_add(out=den, in0=ebc,
                                                    scalar1=0.0)
                    else:
                        nc.vector.tensor_tensor(
                            out=den, in0=den, in1=ebc,
                            op=mybir.AluOpType.add,
                        )
                    # vp_kk = e_bc (bcast over l) * v_shift
                    vp = vpp.tile([P, PT, HW], BF16, tag="vp")
                    nc.vector.tensor_tensor(
                        out=vp.rearrange("p l (a b) -> p l a b", a=H),
                        in0=vpad[:, :, di:di + H, dj:dj + W],
                        in1=bass.AP(
                            tensor=ebc.tensor, offset=ebc.offset,
                            ap=[list(ebc.ap[0]), [0, PT], [W, H], [1, W]],
                        ),
                        op=mybir.AluOpType.mult,
                    )
                    # acc_l += vp[:, l, :] (identity-matmul PSUM accumulation)
                    for l in range(PT):
                        for n in range(NHALF):
                            nc.tensor.matmul(
                                accs[l][:, _ns(n)], ident, vp[:, l, _ns(n)],
                                start=(kk == 0), stop=(kk == NKK - 1),
                                skip_group_check=True,
                            )

                denf = attp.tile([P, HW], F32, tag="denf")
                nc.scalar.activation(out=denf, in_=den,
                                     func=mybir.ActivationFunctionType.Identity)
                recip = attp.tile([P, HW], F32, tag="recip")
                nc.vector.reciprocal_approx_fast(out=recip, in_=denf)

                # ---- h2 = relu(acc * recip + bnatt_b) ----
                h2 = actp.tile([P, PT, HW], BF16, tag="h2")
                for l in range(PT):
                    t3 = tmpp.tile([P, HW], BF16, tag="t3")
                    nc.vector.tensor_tensor(
                        out=t3, in0=accs[l], in1=recip, op=mybir.AluOpType.mult,
                    )
                    nc.scalar.activation(
                        out=h2[:, l, :], in_=t3,
                        func=mybir.ActivationFunctionType.Relu,
                        bias=batt[:, l:l + 1], scale=1.0,
                    )

                # ---- conv3 + residual (identity matmul on bf16 x) + relu ----
                outb = outzp.tile([P, OC, HW], BF16, tag="outb")
                for oc in range(OC):
                    for n in range(NHALF):
                        ps = pmm.tile([P, 512], F32, tag="mm")
                        for kc in range(PT):
                            nc.tensor.matmul(
                                ps,
                                w3T[:, kc, oc * P:(oc + 1) * P],
                                h2[:, kc, _ns(n)],
                                start=(kc == 0), stop=False,
                                skip_group_check=True,
                            )
                        nc.tensor.matmul(
                            ps, ident, xb[:, oc, _ns(n)],
                            start=False, stop=True,
                            skip_group_check=True,
                        )
                        nc.scalar.activation(
                            out=outb[:, oc, _ns(n)], in_=ps,
                            func=mybir.ActivationFunctionType.Relu,
                            bias=b3[:, oc], scale=1.0,
                        )
                    if oc % 2 == 1:
                        nc.scalar.dma_start(
                            out=out_d[b, :, (oc - 1) * HW:(oc + 1) * HW],
                            in_=outb[:, oc - 1:oc + 1, :]
                                .rearrange("p k m -> p (k m)"))

    nc.compile()
    return nc


_PROG = None


def _chan_order():
    # chan(p, l) = 8*(p//4) + 2*(p%4) + l
    order = np.zeros((P, PT), np.int64)
    for p in range(P):
        for l in range(PT):
            order[p, l] = 8 * (p // 4) + 2 * (p % 4) + l
    return order


def _host_prep(inputs):
    import ml_dtypes
    bf = ml_dtypes.bfloat16
    f = lambda a: np.asarray(a, dtype=np.float32)
    x = f(inputs["x"])
    # fold bn scales into weights (bn(conv(x,W),s,b) = conv(x, s*W) + b)
    w1 = f(inputs["w_conv1"]) * f(inputs["bn1_s"])[:, None]
    wq = f(inputs["wq"]) * f(inputs["bnq_s"])[:, None]
    wk = f(inputs["wk"]) * f(inputs["bnk_s"])[:, None]
    # fold bnatt scale through the (linear) attention-value path into v
    sv = f(inputs["bnatt_s"]) * f(inputs["bnv_s"])
    wv = f(inputs["wv"]) * sv[:, None]
    bv = f(inputs["bnatt_s"]) * f(inputs["bnv_b"])
    w3 = f(inputs["w_conv3"]) * f(inputs["bn3_s"])[:, None]

    posf = (f(inputs["pos_h"]) + f(inputs["pos_w"])).reshape(WIDTH, NKK)
    ordr = _chan_order()                                  # [128, 2] channel ids

    def qkvT(w):
        # lhsT chunks: [kc, p(contraction over h1), l, cols=chan(p',l)]
        wT = w.T.reshape(PT, P, WIDTH)                    # [kc, p, cout]
        out = np.zeros((PT, P, PT, P), np.float32)
        for l in range(PT):
            out[:, :, l, :] = wT[:, :, ordr[:, l]]
        return out.astype(bf)

    def bias2(vec):
        o = np.zeros((P, PT), np.float32)
        for l in range(PT):
            o[:, l] = vec[ordr[:, l]]
        return o

    # conv3 lhsT: contraction rows are h2 channels in interleaved order
    w3T = np.zeros((PT, P, OUT), np.float32)
    for l in range(PT):
        w3T[l] = w3.T[ordr[:, l], :]

    pos2 = np.zeros((P, PT, NKK), np.float32)
    for l in range(PT):
        pos2[:, l, :] = posf[ordr[:, l], :]

    s4 = np.zeros((P, P), np.float32)
    for p in range(P):
        for p2 in range(P):
            if p // 4 == p2 // 4:
                s4[p, p2] = 1.0

    com = {
        "w1T": np.ascontiguousarray(w1.T.reshape(KC1, P, WIDTH)).astype(bf),
        "wqT": qkvT(wq),
        "wkT": qkvT(wk),
        "wvT": qkvT(wv),
        "w3T": np.ascontiguousarray(w3T).astype(bf),
        "b1": f(inputs["bn1_b"]).reshape(PT, P, 1),
        "bq": bias2(f(inputs["bnq_b"])),
        "bk": bias2(f(inputs["bnk_b"])),
        "bv": bias2(bv),
        "batt": bias2(f(inputs["bnatt_b"])),
        "b3": f(inputs["bn3_b"]).reshape(OC, P, 1),
        "pos2": pos2,
        "s4": s4.astype(bf),
        "ident": np.eye(P, dtype=np.float32).astype(bf),
    }
    # x p-major: [BL, p, kc*hw]
    xr = x.reshape(B, KC1, P, HW).transpose(0, 2, 1, 3).reshape(B, P, KC1 * HW)
    in_maps = []
    for c in range(NC_):
        xs = np.ascontiguousarray(xr[c * BL:(c + 1) * BL])
        in_maps.append(dict(com, x16=xs.astype(bf)))
    return in_maps


def kernel(**inputs):
    global _PROG
    if _PROG is None:
        _PROG = build_program()
    in_maps = _host_prep(inputs)
    res = run_bass_kernel_spmd(_PROG, in_maps, core_ids=list(range(NC_)))
    outs = []
    for c in range(NC_):
        o = res.results[c]["out"].astype(np.float32)      # [BL, P, OC*HW]
        o = o.reshape(BL, P, OC, HW).transpose(0, 2, 1, 3).reshape(BL, OUT, H, W)
        outs.append(o)
    return np.concatenate(outs, axis=0)
